# revision 30
# baseline (speedup 1.0000x reference)
"""Trainium2 Bass kernel for nn_Attention_54391465836966.

The reference's .reshape calls are RAW byte reinterpretations: token matrix
T = content_feat[b] bytes viewed [S, C] (not a transpose), and s (token-major
[S, C]) is viewed [C, S] before the 1x1 conv.  The host passes every input
pre-arranged into its exact SBUF image (one [128, X] contiguous DMA each, in
bf16), with the token views pre-transposed to channel-major, so the device
does no PE transposes; the s view is realized with SBUF->SBUF DMAs that
re-pair token rows (s2d[r] = tokens (2r, 2r+1) concatenated).

Per core (b = core//4, n = core%4), channel-major [C, S] throughout:
  ctok = cfT + posT ; ctmp = compT + posT
  qT = Wq^T ctok ; kT = Wkv[:, :C]^T ctmp ; v = ctmp^T Wkv[:, C:]
  per head h: P = exp(scale k_h^T q); o_h = (v_h^T P) / Z   (Z via ones col)
  s_tok = packed^T Wproj                                     (token-major)
  const (token-quarter n, full scale): s_cq = ctokQ^T Wproj + bproj
  out_p = WconvT[:C]^T s2d + WconvT[quarter]^T s2d_cq + bconv/4
  out_cf = WconvT[C:, out-quarter]^T cf_raw                  (host-placed)
Host sums the 4 component partials per batch and places out_cf quarter rows.
The affine const terms are distributed so no gated-zero work exists.

Dtypes: bf16 throughout (PE rate = fp32r, half the DMA/SBUF traffic; DVE
adds get the 2x mode); PSUM and the softmax-normalization scratch stay f32.

Schedule: attention is ACT(exp)-bound at ~1.2us/kt, so only v and the
(kT, qT) pair for head pair 0 are computed up front; everything else that
does not gate the exp stream — the remaining k/q groups, the const paths,
and the per-head-pair proj partial sums — is emitted INTO the head loop to
fill PE slack under the exps.  Only the last pair's proj round, the s2d
re-pair, and the conv remain in the tail.  The z-scratch is double-buffered
by head parity; head 6 (cheap even-parity norm) is processed last.
"""
import sys

sys.path.insert(0, "/opt/trn_rl_repo")

import numpy as np

N_CORES = 8
B, C, H, W = 2, 512, 32, 32
S = H * W  # 1024
NH, HD = 8, 64
SCALE = HD ** -0.5

_CACHE = {}


def _img(x, cols):
    """[512, cols] matrix -> its [128, 4*cols] SBUF image (4 row-blocks
    side by side), in bf16."""
    import ml_dtypes
    return np.ascontiguousarray(
        x.reshape(4, 128, cols).transpose(1, 0, 2).reshape(128, 4 * cols)
    ).astype(ml_dtypes.bfloat16)


def _build():
    if "nc" in _CACHE:
        return _CACHE["nc"]
    from contextlib import ExitStack

    import concourse.bacc as bacc
    import concourse.mybir as mybir
    import concourse.tile as tile

    f32 = mybir.dt.float32
    bf16 = mybir.dt.bfloat16
    EXP = mybir.ActivationFunctionType.Exp

    nc = bacc.Bacc("TRN2", target_bir_lowering=False, debug=False,
                   num_devices=N_CORES)

    din = lambda n, s: nc.dram_tensor(n, s, mybir.dt.bfloat16,
                                      kind="ExternalInput").ap()
    pos_d = din("pos", [128, 4096])      # posT image
    cmp_d = din("cmp", [128, 4096])      # compT image
    cft_d = din("cft", [128, 4096])      # cfT image
    cfr_d = din("cfr", [128, 4096])      # raw content_feat[b] image
    wk_d = din("wk", [128, 2048])        # Wkv[:, :C] image
    wv_d = din("wv", [128, 2048])        # Wkv[:, C:] image
    wq_d = din("wq", [128, 2048])        # Wq image
    wproj_d = din("wproj", [128, 2048])  # Wproj image
    wcvs_d = din("wcvs", [128, 2048])    # WconvT[:C] image
    wcvcq_d = din("wcvcq", [128, 512])   # WconvT[C:, out-quarter] image
    wcvsq_d = din("wcvsq", [128, 512])   # WconvT[128n:128(n+1), :]
    cftq_d = din("cftq", [128, 1024])    # cfT[:, token-quarter] image
    posq_d = din("posq", [128, 1024])    # posT[:, token-quarter] image
    bias_d = din("bias2", [1, 1024])     # [bproj, bconv/4]
    out_p = nc.dram_tensor("out_p", [C, S], bf16, kind="ExternalOutput").ap()
    out_cf = nc.dram_tensor("out_cf", [128, S], bf16,
                            kind="ExternalOutput").ap()

    with tile.TileContext(nc) as tc, ExitStack() as ctx:
        main = ctx.enter_context(tc.tile_pool(name="main", bufs=1))

        ones = main.tile([1, 512], bf16, tag="ones")
        nc.gpsimd.memset(ones[:], 1.0)

        # ---- front-critical DMAs: k/q then v path ----
        pos_sb = [main.tile([128, 1024], bf16, tag=f"pos{j}", name=f"pos{j}")
                  for j in range(4)]
        cmp_sb = [main.tile([128, 1024], bf16, tag=f"cmp{j}", name=f"cmp{j}")
                  for j in range(4)]
        cft_sb = [main.tile([128, 1024], bf16, tag=f"cft{j}", name=f"cft{j}")
                  for j in range(4)]
        wv_sb = main.tile([128, 2048], bf16, tag="wv")
        wk_sb = main.tile([128, 2048], bf16, tag="wk")
        wq_sb = main.tile([128, 2048], bf16, tag="wq")
        ctok = [main.tile([128, S], bf16, tag=f"ctk{j}", name=f"ctok{j}")
                for j in range(4)]
        ctmp = [main.tile([128, S], bf16, tag=f"ct{j}", name=f"ctmp{j}")
                for j in range(4)]
        nc.sync.dma_start(wk_sb[:, 0:1024], wk_d[:, 0:1024])
        for j in range(4):
            nc.sync.dma_start(pos_sb[j][:], pos_d[:, 1024 * j:1024 * (j + 1)])
            nc.sync.dma_start(cmp_sb[j][:], cmp_d[:, 1024 * j:1024 * (j + 1)])
            if j == 0:
                nc.sync.dma_start(wk_sb[:, 1024:2048], wk_d[:, 1024:2048])
            if j == 1:
                nc.sync.dma_start(wv_sb[:, 0:1024], wv_d[:, 0:1024])
            if j == 2:
                nc.sync.dma_start(wv_sb[:, 1024:2048], wv_d[:, 1024:2048])
            nc.vector.tensor_add(ctmp[j][:], cmp_sb[j][:], pos_sb[j][:])
        for j in range(4):
            nc.sync.dma_start(cft_sb[j][:], cft_d[:, 1024 * j:1024 * (j + 1)])
            if j == 0:
                nc.sync.dma_start(wq_sb[:], wq_d[:])
            nc.gpsimd.tensor_add(ctok[j][:], cft_sb[j][:], pos_sb[j][:])

        # ---- late weights / const-path inputs ----
        cfr_sb = [main.tile([128, 1024], bf16, tag=f"cfr{j}", name=f"cfr{j}")
                  for j in range(4)]
        wproj_sb = main.tile([128, 2048], bf16, tag="wp")
        wcvs_sb = main.tile([128, 2048], bf16, tag="wcs")
        wcvcq_sb = main.tile([128, 512], bf16, tag="wcc")
        wcvsq_sb = main.tile([128, 512], bf16, tag="wcsq")
        cftq_sb = main.tile([128, 1024], bf16, tag="cftq")
        posq_sb = main.tile([128, 1024], bf16, tag="posq")
        bias_sb = main.tile([1, 1024], bf16, tag="bias")
        nc.sync.dma_start(wproj_sb[:], wproj_d[:])
        for j in range(4):
            nc.sync.dma_start(cfr_sb[j][:], cfr_d[:, 1024 * j:1024 * (j + 1)])
        nc.sync.dma_start(wcvcq_sb[:], wcvcq_d[:])
        nc.sync.dma_start(cftq_sb[:], cftq_d[:])
        nc.sync.dma_start(posq_sb[:], posq_d[:])
        nc.sync.dma_start(bias_sb[:], bias_d[:])
        nc.sync.dma_start(wcvs_sb[:], wcvs_d[:])
        nc.sync.dma_start(wcvsq_sb[:], wcvsq_d[:])

        # norm scratch (x2 by parity, shared zs2) + late tiles
        zraw = [main.tile([1, S], f32, tag=f"zraw{i}", name=f"zraw{i}")
                for i in range(2)]
        zs2 = main.tile([1, S], f32, tag="zs2")
        zinv = [main.tile([1, S], f32, tag=f"zinv{i}", name=f"zinv{i}")
                for i in range(2)]
        zbc = [main.tile([128, S], f32, tag=f"zbc{i}", name=f"zbc{i}")
               for i in range(2)]
        ocf_sb = main.tile([128, S], bf16, tag="cfr0", name="ocf")
        ctokq = main.tile([128, 1024], bf16, tag="ctokq")
        packed = [main.tile([128, S], bf16, tag=f"pk{j}", name=f"pk{j}")
                  for j in range(4)]
        outp = [main.tile([128, S], bf16, tag=f"op{j}", name=f"op{j}")
                for j in range(4)]
        kT = [main.tile([128, S], bf16, tag=f"kT{j}", name=f"kT{j}")
              for j in range(4)]
        qT = [main.tile([128, S], bf16, tag=f"qT{j}", name=f"qT{j}")
              for j in range(4)]
        # v pairs: [p, (i, 128h + [64 d | Z | 63 pad])], pads/Z = 1.0
        v_sb = [main.tile([128, 2048], bf16, tag=f"v{t}", name=f"v{t}")
                for t in range(4)]
        # proj partial accumulators (f32) ride the dead pos/cmp slots;
        # final s tiles ride the dead cft slots
        s_acc = [main.tile([128, 512], f32,
                           tag=(f"pos{i}" if i < 4 else f"cmp{i - 4}"),
                           name=f"sa{i}") for i in range(8)]
        s_sb = [main.tile([128, 512], bf16, tag=f"cft{i % 4}", name=f"s{i}")
                for i in range(8)]
        s2d = [main.tile([128, S], bf16, tag=f"s2d{i}", name=f"s2d{i}")
               for i in range(4)]

        def kq_part(psum, j, part):
            """One of four (kT/qT, qc) projection groups for head-pair j."""
            kind, qc = part % 2, part // 2
            w, act, dst = ((wk_sb, ctmp, kT) if kind == 0
                           else (wq_sb, ctok, qT))
            acc = psum.tile([128, 512], f32, tag="mm")
            for k in range(4):
                nc.tensor.matmul(
                    acc[:],
                    w[:, 512 * k + 128 * j:512 * k + 128 * (j + 1)],
                    act[k][:, 512 * qc:512 * (qc + 1)],
                    start=(k == 0), stop=(k == 3))
            nc.vector.tensor_copy(dst[j][:, 512 * qc:512 * (qc + 1)], acc[:])

        with tc.tile_pool(name="psA", bufs=2, space="PSUM") as ps:
            for part in range(4):
                kq_part(ps, 0, part)
            # ---- v ----
            for t in range(4):
                nc.gpsimd.memset(v_sb[t][:], 1.0)
            for kt in range(8):
                acc = ps.tile([128, 512], f32, tag="mm")
                for k in range(4):
                    nc.tensor.matmul(acc[:],
                                     ctmp[k][:, 128 * kt:128 * (kt + 1)],
                                     wv_sb[:, 512 * k:512 * (k + 1)],
                                     start=(k == 0), stop=(k == 3))
                dst = v_sb[kt // 2][:, 1024 * (kt % 2):1024 * (kt % 2) + 1024]
                nc.scalar.copy(
                    dst.rearrange("p (m c) -> p m c", m=8)[:, :, 0:64],
                    acc[:].rearrange("p (m c) -> p m c", m=8))

        # ---- attention, with remaining work streamed into PE slack ----
        with tc.tile_pool(name="psS", bufs=2, space="PSUM") as psS, \
             tc.tile_pool(name="psO", bufs=1, space="PSUM") as psO, \
             tc.tile_pool(name="psT", bufs=2, space="PSUM") as psT:
            ptp = [main.tile([128, 2048], bf16, tag=f"pt{t}", name=f"pt{t}")
                   for t in range(4)]
            s_cq = [main.tile([128, 512], bf16, tag=f"scq{i}", name=f"scq{i}")
                    for i in range(2)]
            s2d_cq = main.tile([128, S], bf16, tag="s2dcq")

            def scq_part(i):
                if i == 0:
                    nc.gpsimd.tensor_add(ctokq[:], cftq_sb[:], posq_sb[:])
                acc = psT.tile([128, 512], f32, tag="mm")
                nc.tensor.matmul(acc[:], ones[0:1, 0:128],
                                 bias_sb[0:1, 0:512], start=True, stop=False)
                for a in range(4):
                    nc.tensor.matmul(
                        acc[:],
                        ctokq[:, 256 * a + 128 * i:256 * a + 128 * (i + 1)],
                        wproj_sb[:, 512 * a:512 * (a + 1)],
                        start=False, stop=(a == 3))
                nc.vector.tensor_copy(s_cq[i][:], acc[:])
                if i == 1:
                    for g in range(2):
                        for sh in range(2):
                            nc.sync.dma_start(
                                s2d_cq[64 * sh:64 * sh + 64,
                                       512 * g:512 * g + 512],
                                s_cq[sh][g:128:2, :])

            def ocf_part(half):
                acc = psT.tile([128, 512], f32, tag="mm")
                for k in range(4):
                    nc.tensor.matmul(
                        acc[:], wcvcq_sb[:, 128 * k:128 * (k + 1)],
                        cfr_sb[k][:, 512 * half:512 * (half + 1)],
                        start=(k == 0), stop=(k == 3))
                nc.vector.tensor_copy(
                    ocf_sb[:, 512 * half:512 * (half + 1)], acc[:])
                if half == 1:
                    nc.sync.dma_start(out_cf[:, :], ocf_sb[:])

            def proj_round(j, u, last=False):
                """Proj partial for head-pair j, token tiles 2u, 2u+1."""
                for t in (2 * u, 2 * u + 1):
                    acc = psT.tile([128, 512], f32, tag="mm")
                    nc.tensor.matmul(acc[:],
                                     packed[j][:, 128 * t:128 * (t + 1)],
                                     wproj_sb[:, 512 * j:512 * (j + 1)],
                                     start=True, stop=True)
                    if j == 0:
                        nc.vector.tensor_copy(s_acc[t][:], acc[:])
                    elif not last:
                        nc.vector.tensor_add(s_acc[t][:], s_acc[t][:],
                                             acc[:])
                    else:
                        nc.vector.tensor_add(s_sb[t][:], s_acc[t][:], acc[:])
                if last:
                    for g in range(2):
                        for sh in range(2):
                            nc.sync.dma_start(
                                s2d[u][64 * sh:64 * sh + 64,
                                       512 * g:512 * g + 512],
                                s_sb[2 * u + sh][g:128:2, :])

            filler = {}
            for hh in range(3):  # k/q groups for pairs 1..3
                for sl_i in range(4):
                    filler[hh, 2 * sl_i + 1] = (
                        lambda hh=hh, sl_i=sl_i: kq_part(psT, hh + 1, sl_i))
            for hh, j in ((3, 0), (4, 1), (6, 2)):  # proj rounds 0..2
                for u in range(4):
                    filler[hh, 2 * u + 1] = (
                        lambda j=j, u=u: proj_round(j, u))
            filler[5, 1] = lambda: scq_part(0)
            filler[5, 3] = lambda: scq_part(1)
            filler[5, 5] = lambda: ocf_part(0)
            filler[5, 7] = lambda: ocf_part(1)

            # head 6 (even parity: short norm chain) goes LAST so the final
            # normalization before the proj tail is the cheap direct-mul one
            for hi, h in enumerate([0, 1, 2, 3, 4, 5, 7, 6]):
                j, row = h // 2, 64 * (h % 2)
                o_ps = psO.tile([128, S], f32, tag="o")
                pend = []  # PV runs one kt-pair behind to hide slot waits
                for kt in range(8):
                    sc = psS.tile([128, S], f32, tag="sc")
                    for qc in range(2):
                        nc.tensor.matmul(
                            sc[:, 512 * qc:512 * (qc + 1)],
                            kT[j][row:row + 64, 128 * kt:128 * (kt + 1)],
                            qT[j][row:row + 64, 512 * qc:512 * (qc + 1)],
                            start=True, stop=True)
                    nc.scalar.activation(
                        ptp[kt // 2][:, 1024 * (kt % 2):1024 * (kt % 2) + 1024],
                        sc[:], EXP, scale=SCALE)
                    if kt % 2 == 1:
                        pend.append(kt // 2)
                    todo = []
                    if len(pend) > 1:
                        todo = [pend.pop(0)]
                    if kt == 7:
                        todo = todo + pend
                    for ktp in todo:
                        vv = v_sb[ktp][:].rearrange("p (i c) -> p i c", i=2)
                        pp = ptp[ktp][:].rearrange("p (i c) -> p i c", i=2)
                        for i in range(2):
                            for qc in range(2):
                                nc.tensor.matmul(
                                    o_ps[:, 512 * qc:512 * (qc + 1)],
                                    vv[:, i, 128 * h:128 * (h + 1)],
                                    pp[:, i, 512 * qc:512 * (qc + 1)],
                                    start=(ktp == 0 and i == 0),
                                    stop=(ktp == 3 and i == 1))
                    f = filler.get((hi, kt))
                    if f is not None:
                        f()
                # normalization: Z row -> recip -> broadcast -> scale
                par = hi % 2
                nc.vector.tensor_copy(zraw[par][0:1, :], o_ps[64:65, :])
                nc.vector.reciprocal_approx_accurate(
                    zinv[par][0:1, :], zraw[par][0:1, :], zs2[0:1, :])
                nc.gpsimd.partition_broadcast(zbc[par][:], zinv[par][0:1, :])
                rows = slice(64 * (h % 2), 64 * (h % 2) + 64)
                if h % 2 == 0:
                    nc.vector.tensor_mul(packed[j][rows, :], o_ps[0:64, :],
                                         zbc[par][rows, :])
                else:
                    nc.vector.tensor_copy(packed[j][rows, :], o_ps[0:64, :])
                    nc.gpsimd.tensor_mul(packed[j][rows, :],
                                         packed[j][rows, :],
                                         zbc[par][rows, :])

            # ---- tail: last proj round -> s2d -> conv ----
            for u in range(4):
                proj_round(3, u, last=True)
            for oc in range(4):
                for half in range(2):
                    acc = psT.tile([128, 512], f32, tag="mm")
                    nc.tensor.matmul(
                        acc[:],
                        bias_sb[0:1, 512 + 128 * oc:512 + 128 * (oc + 1)],
                        ones[0:1, :], start=True, stop=False)
                    for r in range(4):
                        nc.tensor.matmul(
                            acc[:],
                            wcvs_sb[:, 512 * r + 128 * oc:
                                    512 * r + 128 * (oc + 1)],
                            s2d[r][:, 512 * half:512 * (half + 1)],
                            start=False, stop=False)
                    nc.tensor.matmul(
                        acc[:], wcvsq_sb[:, 128 * oc:128 * (oc + 1)],
                        s2d_cq[:, 512 * half:512 * (half + 1)],
                        start=False, stop=True)
                    eng = nc.vector if (oc + half) % 2 == 0 else nc.scalar
                    if eng is nc.scalar:
                        eng.copy(outp[oc][:, 512 * half:512 * (half + 1)],
                                 acc[:])
                    else:
                        eng.tensor_copy(
                            outp[oc][:, 512 * half:512 * (half + 1)], acc[:])
                    nc.sync.dma_start(
                        out_p[128 * oc:128 * (oc + 1),
                              512 * half:512 * (half + 1)],
                        outp[oc][:, 512 * half:512 * (half + 1)])

    nc.compile()
    _CACHE["nc"] = nc
    return nc


def _shard_inputs(content_feat, components, pos_emb, Wq, Wkv, Wproj, bproj,
                  Wconv, bconv):
    import ml_dtypes
    bf = ml_dtypes.bfloat16
    f = np.float32
    posT = pos_emb.reshape(S, C).T.astype(f)
    pos_img = _img(posT, S)
    wconvT = Wconv.T.astype(f)                        # [2C, C]
    wk_img = _img(np.ascontiguousarray(Wkv[:, :C]), C)
    wv_img = _img(np.ascontiguousarray(Wkv[:, C:]), C)
    wq_img = _img(Wq, C)
    wproj_img = _img(Wproj, C)
    wcvs_img = _img(np.ascontiguousarray(wconvT[:C]), C)
    bias2 = np.ascontiguousarray(
        np.concatenate([bproj, bconv / 4]).reshape(1, 1024)).astype(bf)
    in_maps = []
    for core in range(N_CORES):
        b, n = core // 4, core % 4
        sl = slice(128 * n, 128 * (n + 1))
        tq = slice(256 * n, 256 * (n + 1))
        cfT = np.ascontiguousarray(content_feat[b].reshape(S, C).T)
        in_maps.append({
            "pos": pos_img,
            "cmp": _img(components[n, b].reshape(S, C).T, S),
            "cft": _img(cfT, S),
            "cfr": _img(content_feat[b].reshape(C, S), S),
            "wk": wk_img,
            "wv": wv_img,
            "wq": wq_img,
            "wproj": wproj_img,
            "wcvs": wcvs_img,
            "wcvcq": _img(np.ascontiguousarray(wconvT[C:, sl]), 128),
            "wcvsq": np.ascontiguousarray(wconvT[sl]).astype(bf),
            "cftq": _img(np.ascontiguousarray(cfT[:, tq]), 256),
            "posq": _img(np.ascontiguousarray(posT[:, tq]), 256),
            "bias2": bias2,
        })
    return in_maps


def _run(trace=False, **inputs):
    from concourse.bass_utils import run_bass_kernel_spmd

    nc = _build()
    in_maps = _shard_inputs(**inputs)
    res = run_bass_kernel_spmd(nc, in_maps, list(range(N_CORES)), trace=trace)
    full = np.empty((B, C, S), dtype=np.float32)
    for b in range(B):
        acc = sum(res.results[4 * b + n]["out_p"].astype(np.float32)
                  for n in range(4))
        for n in range(4):
            acc[128 * n:128 * (n + 1)] += \
                res.results[4 * b + n]["out_cf"].astype(np.float32)
        full[b] = acc
    return full.reshape(B, C, H, W).astype(np.float32), res


def kernel(**inputs):
    out, _ = _run(trace=False, **inputs)
    return out
